# revision 45
# baseline (speedup 1.0000x reference)
"""Trainium2 Bass kernel for nn_CrossEntropyLoss_71133248356852.

Computes, for full inputs (B=2M rows, C=10):
    e   = P_exp(x)            (deg-8 poly, Horner in the reference)
    s   = rowsum(e)
    inv = P_inv(s), then `iterations` Newton-Raphson steps toward 1/s
    u   = e * inv             (softmax)
    out = -sum(t * P_log(u)) / B

Device strategy (pure data parallel over 8 cores, batch-dim sharded):
  Each degree-8 polynomial is factored on the host (np.roots) into 4 real
  monic quadratics:  P(x)/c8 = prod_i [(x+a_i)^2 + b_i]  (exact in real
  arithmetic; conjugate root pairs).  On-device each quadratic is ONE
  ScalarEngine ACT-Square op (free affine: Square(scale*x+bias)), the +b_i
  folds into fused scalar_tensor_tensor products on DVE/Pool.

  Leading coefficients are folded algebraically instead of multiplied:
    e' = P_exp/c8, s' = s/c8.  The inverse-poly coeffs are host-adapted
    (a_j <- a_j * c8^(j+1)) so h = c8*inv follows the SAME NR recurrence
    with s'.  Then e'*h == e*inv exactly.  NR runs in negated space
    (h' = -h: h' <- (s'*h' + 2)*h') because the ALU has no reverse-sub;
    the sign folds into the log-stage ACT scale=-1.  P_log's leading
    coefficient d8 becomes a host-side scale of the final scalar.

  The target weights fold into the first log-stage quadratic
  ((lsq1+mu1)*t via one fused scalar_tensor_tensor), and the final product+
  reduction is one custom-DVE affine_mul_reduce into a per-tile per-partition
  accumulator column; the host sums the [128, NT] partials across cores.

  Emission is software-pipelined (stage A: load+exp poly+rowsum, stage B:
  inverse+NR+softmax, stage C: log poly+accumulate) with a multi-tile skew so
  the Tile list scheduler interleaves tiles instead of head-of-line blocking
  each engine on the previous tile's tail.
"""

import sys

for _p in ("/opt/trn_rl_repo",):
    if _p not in sys.path:
        sys.path.insert(0, _p)

import numpy as np

B = 2_000_000
C = 10
N_CORES = 8
# Per-core rows padded to 128*1954 = 250112 (pad rows have target=0 -> no
# contribution). Tiles: rows-per-partition T per tile, sum(TILE_TS) = 1954.
R_CORE = 250_112
TILE_TS = [104] * 18 + [82]
NT = len(TILE_TS)
BUFS_IO = 2
BUFS_WK = 2
BUFS_SM = 3
SKEW_B = 1
SKEW_C = 5

_KERNEL_CACHE = {}


def _pair_quadratics(coeffs):
    """coeffs ascending, degree 8. Returns 4 (a, b) pairs with
    prod_i ((x+a_i)^2 + b_i) == p(x)/coeffs[-1], or None if it can't be
    done in a numerically trustworthy way."""
    c = np.asarray(coeffs, np.float64)
    if c[-1] == 0.0 or not np.all(np.isfinite(c)):
        return None
    r = np.roots(c[::-1])
    if len(r) != 8 or not np.all(np.isfinite(r)):
        return None
    # Split into complex-conjugate pairs and reals.
    tol = 1e-9 * np.maximum(1.0, np.abs(r))
    reals = sorted([z.real for z in r if abs(z.imag) <= tol[0] or abs(z.imag) <= abs(z) * 1e-9])
    cplx = [z for z in r if not (abs(z.imag) <= tol[0] or abs(z.imag) <= abs(z) * 1e-9)]
    pairs = []
    # complex: greedily match conjugates
    cplx_pos = sorted([z for z in cplx if z.imag > 0], key=lambda z: (z.real, z.imag))
    cplx_neg = [z for z in cplx if z.imag <= 0]
    if len(cplx_pos) * 2 != len(cplx) or len(reals) % 2 != 0:
        return None
    for z in cplx_pos:
        j = int(np.argmin([abs(w - np.conj(z)) for w in cplx_neg]))
        w = cplx_neg.pop(j)
        ssum = (z + w).real
        prod = (z * w).real
        pairs.append((-ssum / 2.0, prod - ssum * ssum / 4.0))
    for k in range(0, len(reals), 2):
        z, w = reals[k], reals[k + 1]
        ssum = z + w
        prod = z * w
        pairs.append((-ssum / 2.0, prod - ssum * ssum / 4.0))
    if len(pairs) != 4:
        return None
    return pairs


def _check_factorization(coeffs, pairs, lo, hi):
    """Max relative deviation of the factored form vs float64 Horner on a
    grid, relative to the max |p| scale."""
    c = np.asarray(coeffs, np.float64)
    x = np.linspace(lo, hi, 4097, dtype=np.float64)
    ref = np.polyval(c[::-1], x)
    fac = np.ones_like(x)
    for a, b in pairs:
        fac = fac * ((x + a) ** 2 + b)
    fac = fac * c[-1]
    scale = np.max(np.abs(ref)) + 1e-300
    return float(np.max(np.abs(fac - ref)) / scale)


def _host_reference(enc_input, enc_target, exp_coeffs, inverse_coeffs, log_coeffs, iterations):
    """Exact reference semantics on host (fallback path)."""
    def pv(cs, v):
        r = np.full_like(v, cs[-1])
        for i in range(len(cs) - 2, -1, -1):
            r = r * v + cs[i]
        return r

    x = enc_input.astype(np.float32)
    t = enc_target.astype(np.float32)
    e = pv(exp_coeffs.astype(np.float32), x)
    s = e.sum(axis=1, keepdims=True, dtype=np.float32)
    inv = pv(inverse_coeffs.astype(np.float32), s)
    for _ in range(int(iterations)):
        inv = inv * (np.float32(2.0) - s * inv)
    u = e * inv
    ls = pv(log_coeffs.astype(np.float32), u)
    return np.float32(-(t * ls).sum(dtype=np.float32) / x.shape[0])


def _build_nc(pe, pl, g, n_iters, tile_ts=None, bufs_io=3, bufs_wk=2, bufs_sm=3,
              skew_b=1, skew_c=2):
    """Build the Bass program. pe/pl: 4 (a,b) quadratic pairs for the exp/log
    polys; g: 5 ascending coeffs of the NEGATED adapted inverse poly."""
    import concourse.bacc as bacc
    import concourse.tile as tile
    import concourse.mybir as mybir

    if tile_ts is None:
        tile_ts = TILE_TS
    assert sum(tile_ts) * 128 == R_CORE
    nt = len(tile_ts)

    f32 = mybir.dt.float32
    Alu = mybir.AluOpType
    Act = mybir.ActivationFunctionType
    AxX = mybir.AxisListType.X

    nc = bacc.Bacc("TRN2", target_bir_lowering=False, debug=False)
    x_d = nc.dram_tensor("x", [R_CORE, C], f32, kind="ExternalInput").ap()
    t_d = nc.dram_tensor("t", [R_CORE, C], f32, kind="ExternalInput").ap()
    cb_d = nc.dram_tensor("cb", [128, 8], f32, kind="ExternalInput").ap()
    acc_d = nc.dram_tensor("acc", [128, nt], f32, kind="ExternalOutput").ap()

    with tile.TileContext(nc) as tc:
        with (
            tc.tile_pool(name="io", bufs=bufs_io) as io,
            tc.tile_pool(name="work", bufs=bufs_wk) as wk,
            tc.tile_pool(name="small", bufs=bufs_sm) as sm,
            tc.tile_pool(name="accp", bufs=1) as accp,
        ):
            acc = accp.tile([128, nt], f32, tag="acc")
            # ACT Square requires bias as an SBUF AP: the 8 quadratic shifts
            # (4 exp + 4 log) come in via one DMA'd const input (a single
            # writer keeps per-instruction sync-wait counts low).
            cb = accp.tile([128, 8], f32, tag="cbias")
            nc.sync.dma_start(cb[:], cb_d)
            # warm the ACT Square table set while the first x tile loads
            warm = accp.tile([128, 1], f32, tag="warm")
            nc.scalar.activation(warm[:], cb[:, 0:1], Act.Square)

            row_starts = []
            r0 = 0
            for T in tile_ts:
                row_starts.append(r0)
                r0 += 128 * T

            st = {}  # per-tile in-flight tiles

            def stage_a(i):
                # load + exp poly + row sums
                T = tile_ts[i]
                F = T * C
                rows = 128 * T
                row0 = row_starts[i]
                xs = x_d[row0:row0 + rows, :].rearrange("(p t) c -> p (t c)", p=128)
                ts_ = t_d[row0:row0 + rows, :].rearrange("(p t) c -> p (t c)", p=128)

                x = io.tile([128, F], f32, tag="x", bufs=bufs_io)
                nc.sync.dma_start(x[:], xs)
                t = io.tile([128, F], f32, tag="t", bufs=skew_c + 1)
                nc.sync.dma_start(t[:], ts_)

                sq = []
                for k in range(4):
                    q = wk.tile([128, F], f32, tag=f"sq{k}", bufs=2, name=f"sq{k}")
                    nc.scalar.activation(q[:], x[:], Act.Square,
                                         bias=cb[:, k:k + 1], scale=1.0)
                    sq.append(q)
                a2 = wk.tile([128, F], f32, tag="a2", bufs=3, name="a2")
                nc.gpsimd.tensor_scalar_add(a2[:], sq[1][:], float(pe[1][1]))
                a4 = wk.tile([128, F], f32, tag="a4", bufs=3, name="a4")
                nc.gpsimd.tensor_scalar_add(a4[:], sq[3][:], float(pe[3][1]))
                m1 = wk.tile([128, F], f32, tag="m1", bufs=3, name="m1")
                nc.vector.scalar_tensor_tensor(
                    m1[:], sq[0][:], float(pe[0][1]), a2[:], Alu.add, Alu.mult)
                m2 = wk.tile([128, F], f32, tag="m2", bufs=3, name="m2")
                nc.vector.scalar_tensor_tensor(
                    m2[:], sq[2][:], float(pe[2][1]), a4[:], Alu.add, Alu.mult)
                e = wk.tile([128, F], f32, tag="e", bufs=skew_b + 2, name="e")
                nc.vector.tensor_tensor(e[:], m1[:], m2[:], Alu.mult)

                s = sm.tile([128, T], f32, tag="s", name="s")
                nc.vector.tensor_reduce(
                    s[:], e[:].rearrange("p (t c) -> p t c", c=C), AxX, Alu.add)
                st[i] = {"t": t, "e": e, "s": s}

            def stage_b(i):
                # inverse poly + NR + softmax
                T = tile_ts[i]
                F = T * C
                e, s = st[i]["e"], st[i]["s"]
                y = sm.tile([128, T], f32, tag="y", name="y")
                nc.scalar.activation(y[:], s[:], Act.Square)
                v1 = sm.tile([128, T], f32, tag="v1", name="v1")
                nc.gpsimd.tensor_scalar(
                    v1[:], y[:], float(g[4]), float(g[2]), Alu.mult, Alu.add)
                s3 = sm.tile([128, T], f32, tag="s3", name="s3")
                nc.gpsimd.tensor_scalar(
                    s3[:], s[:], float(g[3]), None, Alu.mult, Alu.bypass)
                v2 = sm.tile([128, T], f32, tag="v2", name="v2")
                nc.gpsimd.tensor_tensor(v2[:], s3[:], v1[:], Alu.add)
                v3 = sm.tile([128, T], f32, tag="v3", name="v3")
                nc.gpsimd.tensor_tensor(v3[:], y[:], v2[:], Alu.mult)
                v4 = sm.tile([128, T], f32, tag="v4", name="v4")
                nc.vector.scalar_tensor_tensor(
                    v4[:], s[:], float(g[1]), v3[:], Alu.mult, Alu.add)
                h = sm.tile([128, T], f32, tag="h", name="h")
                nc.gpsimd.tensor_scalar_add(h[:], v4[:], float(g[0]))

                for _ in range(n_iters):
                    wsm = sm.tile([128, T], f32, tag="wsm", name="wsm")
                    nc.gpsimd.tensor_tensor(wsm[:], s[:], h[:], Alu.mult)
                    h2 = sm.tile([128, T], f32, tag="h", name="h2")
                    nc.vector.scalar_tensor_tensor(
                        h2[:], wsm[:], 2.0, h[:], Alu.add, Alu.mult)
                    h = h2

                u = wk.tile([128, F], f32, tag="u",
                            bufs=(skew_c - skew_b) + 1, name="u")
                nc.gpsimd.tensor_tensor(
                    u[:].rearrange("p (t c) -> p t c", c=C),
                    e[:].rearrange("p (t c) -> p t c", c=C),
                    h[:, :, None].broadcast_to([128, T, C]),
                    Alu.mult)
                st[i]["u"] = u

            def stage_c(i):
                # log poly + target weighting + accumulate
                T = tile_ts[i]
                F = T * C
                t, u = st[i]["t"], st[i]["u"]
                lsq = []
                for k in range(4):
                    q = wk.tile([128, F], f32, tag=f"lsq{k}", bufs=2, name=f"lsq{k}")
                    nc.scalar.activation(q[:], u[:], Act.Square,
                                         bias=cb[:, 4 + k:5 + k], scale=-1.0)
                    lsq.append(q)
                b2t = wk.tile([128, F], f32, tag="a2", bufs=3, name="b2t")
                nc.vector.scalar_tensor_tensor(
                    b2t[:], lsq[1][:], float(pl[1][1]), t[:], Alu.add, Alu.mult)
                b4 = wk.tile([128, F], f32, tag="a4", bufs=3, name="b4")
                nc.gpsimd.tensor_scalar_add(b4[:], lsq[3][:], float(pl[3][1]))
                n1 = wk.tile([128, F], f32, tag="m1", bufs=3, name="n1")
                nc.vector.scalar_tensor_tensor(
                    n1[:], lsq[0][:], float(pl[0][1]), b2t[:], Alu.add, Alu.mult)
                n2 = wk.tile([128, F], f32, tag="m2", bufs=3, name="n2")
                nc.vector.scalar_tensor_tensor(
                    n2[:], lsq[2][:], float(pl[2][1]), b4[:], Alu.add, Alu.mult)

                scr = wk.tile([128, F], f32, tag="e", bufs=skew_b + 2, name="scr")
                nc.vector.affine_mul_reduce(
                    out=scr[:], accum_out=acc[:, i:i + 1], in0=n1[:], in1=n2[:],
                    scale=1.0, bias=0.0)
                del st[i]

            # software-pipelined emission with tile skew: priorities make
            # the list scheduler interleave tiles instead of head-of-line
            # blocking each engine on the previous tile's tail.
            for i in range(nt + skew_c):
                if i < nt:
                    stage_a(i)
                if skew_b <= i and i - skew_b < nt:
                    stage_b(i - skew_b)
                if skew_c <= i and i - skew_c < nt:
                    stage_c(i - skew_c)

            nc.sync.dma_start(acc_d, acc[:])
    nc.compile()
    return nc


def _prep(exp_coeffs, inverse_coeffs, log_coeffs):
    """Host-side coefficient preprocessing. Returns (pe, pl, g, d8) or None."""
    ec = np.asarray(exp_coeffs, np.float64)
    ic = np.asarray(inverse_coeffs, np.float64)
    lc = np.asarray(log_coeffs, np.float64)
    if len(ec) != 9 or len(lc) != 9 or len(ic) != 5:
        return None
    pe = _pair_quadratics(ec)
    pl = _pair_quadratics(lc)
    if pe is None or pl is None:
        return None
    # check on plausible ranges: x in [0,1]; softmax could be anywhere for
    # weird coeffs, use a generous band around [-2, 2].
    if _check_factorization(ec, pe, 0.0, 1.0) > 1e-6:
        return None
    if _check_factorization(lc, pl, -2.0, 2.0) > 1e-6:
        return None
    c8 = ec[-1]
    d8 = lc[-1]
    # negated adapted inverse coeffs: g_j = -ic_j * c8^(j+1)
    g = [-(ic[j] * c8 ** (j + 1)) for j in range(5)]
    if not np.all(np.isfinite(g)):
        return None
    return pe, pl, [float(v) for v in g], float(d8)


def kernel(enc_input, enc_target, exp_coeffs, inverse_coeffs, log_coeffs, iterations):
    enc_input = np.ascontiguousarray(np.asarray(enc_input, np.float32))
    enc_target = np.ascontiguousarray(np.asarray(enc_target, np.float32))
    exp_coeffs = np.asarray(exp_coeffs, np.float32)
    inverse_coeffs = np.asarray(inverse_coeffs, np.float32)
    log_coeffs = np.asarray(log_coeffs, np.float32)
    n_iters = int(np.asarray(iterations))

    assert enc_input.shape == (B, C), enc_input.shape

    prep = _prep(exp_coeffs, inverse_coeffs, log_coeffs)
    if prep is None:
        # Numerically untrustworthy factorization -> exact host fallback.
        return _host_reference(enc_input, enc_target, exp_coeffs,
                               inverse_coeffs, log_coeffs, n_iters)
    pe, pl, g, d8 = prep

    key = (tuple(map(tuple, pe)), tuple(map(tuple, pl)), tuple(g), n_iters)
    nc = _KERNEL_CACHE.get(key)
    if nc is None:
        nc = _build_nc(pe, pl, g, n_iters, tile_ts=TILE_TS,
                       bufs_io=BUFS_IO, bufs_wk=BUFS_WK, bufs_sm=BUFS_SM,
                       skew_b=SKEW_B, skew_c=SKEW_C)
        _KERNEL_CACHE[key] = nc

    # ---- shard + pad ----
    rows_per_core = B // N_CORES  # 250000
    pad = R_CORE - rows_per_core  # 112
    in_maps = []
    for c in range(N_CORES):
        xs = enc_input[c * rows_per_core:(c + 1) * rows_per_core]
        ts = enc_target[c * rows_per_core:(c + 1) * rows_per_core]
        # pad x with a replicated real row (keeps NR dynamics finite exactly
        # when the real data's dynamics are finite); pad t with zeros so the
        # padded rows contribute nothing.
        xp = np.concatenate([xs, np.broadcast_to(xs[0:1], (pad, C))], axis=0)
        tp = np.concatenate([ts, np.zeros((pad, C), np.float32)], axis=0)
        cbv = np.array([pe[k][0] for k in range(4)] + [pl[k][0] for k in range(4)],
                       np.float32)
        in_maps.append({"x": np.ascontiguousarray(xp),
                        "t": np.ascontiguousarray(tp),
                        "cb": np.ascontiguousarray(np.broadcast_to(cbv, (128, 8)))})

    from concourse.bass_utils import run_bass_kernel_spmd
    res = run_bass_kernel_spmd(nc, in_maps, core_ids=list(range(N_CORES)))

    total = np.float64(0.0)
    for r in res.results:
        total += np.float64(r["acc"].astype(np.float64).sum())
    loss = -(d8 * total) / B
    return np.float32(loss)


if __name__ == "__main__":
    pass
